# revision 16
# baseline (speedup 1.0000x reference)
"""Trainium2 Bass kernel for nn_MoE (moe_routing).

Strategy: expert parallelism with host-side dispatch/combine (the
"all-to-all" of the sharding hint, performed while sharding the full
inputs).  The host computes the top-2 gating (cheap: a 4096x1024 @
1024x8 matmul + softmax), gathers each expert's routed tokens into a
padded capacity-C buffer, and core e computes, densely over its C
gathered tokens,

    out_e = gate[t] * (gelu(x_t @ w1[e] + b1[e]) @ w2[e])

The host combine gathers each token's two expert rows and adds the
(gate-weighted) b2 bias terms.  This is exact w.r.t. the reference's
dense-per-expert formulation: every routed (token, expert) pair is
computed; padding rows carry gate 0.

Matmuls run in bf16 (full-rate PE mode, fp32 PSUM accumulation) --
tolerance-checked far under the 2e-2 gate.  Set MOE_F32R_MM=1 for
float32r operands instead (same PE rate at moving-dim >= 256, 2x DMA).
"""

import os
from contextlib import ExitStack

import numpy as np

import concourse.bass as bass
from concourse import bacc
import concourse.mybir as mybir
import concourse.tile as tile
from concourse.bass_utils import run_bass_kernel_spmd

F32 = mybir.dt.float32
BF16 = mybir.dt.bfloat16
F32R = mybir.dt.float32r
AF = mybir.ActivationFunctionType

D_MODEL = 1024
D_HEAD = 2048
N_EXPERTS = 8
TOP_K = 2
N_CORES = 8

DC = D_MODEL // 128      # d_model chunks of 128
HC = D_HEAD // 128       # d_head chunks of 128

LAST_RESULT = None       # BassKernelResults of the most recent run (for test.py)


def _mm_dt():
    return F32R if os.environ.get("MOE_F32R_MM") == "1" else BF16


def _np_mm_dt():
    import ml_dtypes
    return np.float32 if os.environ.get("MOE_F32R_MM") == "1" else ml_dtypes.bfloat16


def _blocks_for(C):
    """Decompose capacity C (multiple of 64) into token blocks of 512
    plus one 64..448 tail."""
    blocks = [512] * (C // 512)
    if C % 512:
        blocks.append(C % 512)
    return blocks


def build_nc(C, reps=1):
    """Build the single-core SPMD Bass program for capacity C tokens."""
    mmdt = _mm_dt()
    nc = bacc.Bacc()
    blocks = _blocks_for(C)
    NQ = -(-C // 128)        # 128-token quarters overall (last may be half)

    xT_ds = [
        nc.declare_dram_parameter(f"xT{bi}", [128, DC, tb], mmdt, isOutput=False)
        for bi, tb in enumerate(blocks)
    ]
    w1_d = nc.declare_dram_parameter("w1", [128, 8, DC, 256], mmdt, isOutput=False)
    w2_d = nc.declare_dram_parameter("w2", [128, 2, HC, 512], mmdt, isOutput=False)
    b1t_d = nc.declare_dram_parameter("b1t", [128, HC], F32, isOutput=False)
    g_d = nc.declare_dram_parameter("g", [128, NQ], F32, isOutput=False)
    out_d = nc.declare_dram_parameter("out", [C, D_MODEL], F32, isOutput=True)

    with tile.TileContext(nc) as tc, ExitStack() as ctx:
        singles = ctx.enter_context(tc.tile_pool(name="singles", bufs=1))
        xt_pool = ctx.enter_context(tc.tile_pool(name="xt", bufs=4))
        ht_pool = ctx.enter_context(tc.tile_pool(name="ht", bufs=3))
        y_pool = ctx.enter_context(tc.tile_pool(name="yb", bufs=4))
        ps_h = ctx.enter_context(tc.tile_pool(name="ps_h", bufs=3, space="PSUM"))
        ps_y = ctx.enter_context(tc.tile_pool(name="ps_y", bufs=2, space="PSUM"))

        b1t_sb = singles.tile([128, HC], F32)
        g_sb = singles.tile([128, NQ], F32)
        nc.gpsimd.dma_start(out=b1t_sb, in_=b1t_d[:])
        nc.gpsimd.dma_start(out=g_sb, in_=g_d[:])

        # w1 as 4 independent hc-group tiles so block 0's mm1 can begin
        # before the full weight load has landed; w2 as 2 dh-half tiles in
        # the order mm2 consumes them.  All weight traffic rides the gpsimd
        # queues so x/y DMAs on nc.sync are never stuck behind it.
        HCG = HC // 8
        w1_sb_g = []
        for gi in range(8):
            t = singles.tile([128, DC, HCG * 128], mmdt, name=f"w1g{gi}")
            w1_sb_g.append(t)
            nc.scalar.dma_start(out=t, in_=w1_d[:, gi])
        w2_sb_g = []
        for dh in range(2):
            t = singles.tile([128, HC, 512], mmdt, name=f"w2g{dh}")
            w2_sb_g.append(t)
            nc.scalar.dma_start(out=t, in_=w2_d[:, dh])

        def emit_prep(blk_i, t0, tb):
            xT = xt_pool.tile([128, DC, tb], mmdt, tag="xT")
            nc.sync.dma_start(out=xT, in_=xT_ds[blk_i][:])
            return xT

        def emit_mm1(xT, tb):
            hT = ht_pool.tile([128, HC, tb], mmdt, tag="hT")
            for hc in range(HC):
                ph = ps_h.tile([128, tb], F32, tag="ph")
                w1t = w1_sb_g[hc // HCG]
                hcl = hc % HCG
                for dc in range(DC):
                    nc.tensor.matmul(
                        ph,
                        lhsT=w1t[:, dc, hcl * 128 : (hcl + 1) * 128],
                        rhs=xT[:, dc],
                        start=(dc == 0),
                        stop=(dc == DC - 1),
                    )
                nc.scalar.activation(
                    hT[:, hc], ph, AF.Gelu, bias=b1t_sb[:, hc : hc + 1]
                )
            return hT

        def tail_mm1_steps(xT, tb, hT):
            """Generator: one tail-mm1 matmul per next() call, so the tail's
            N=64 matmuls (whose weight loads cannot hide behind their own
            21 ns streams) ride between N=512 mm2 matmuls instead."""
            for hc in range(HC):
                ph = ps_h.tile([128, tb], F32, tag="ph")
                w1t = w1_sb_g[hc // HCG]
                hcl = hc % HCG
                for dc in range(DC):
                    nc.tensor.matmul(
                        ph,
                        lhsT=w1t[:, dc, hcl * 128 : (hcl + 1) * 128],
                        rhs=xT[:, dc],
                        start=(dc == 0),
                        stop=(dc == DC - 1),
                    )
                    yield
                nc.scalar.activation(
                    hT[:, hc], ph, AF.Gelu, bias=b1t_sb[:, hc : hc + 1]
                )

        def emit_mm2(t0, tb, hT, inject=None, inject_every=2):
            # quarters of 128 tokens, plus one 64-token half-quarter when
            # tb % 128 == 64 (psum/output partition dim 64).
            quarters = [(q * 128, 128) for q in range(tb // 128)]
            if tb % 128:
                quarters.append((tb - tb % 128, tb % 128))
            n_mm = 0
            for (q0, qn) in quarters:
                qg = (t0 + q0) // 128
                pys = [ps_y.tile([128, 512], F32, name=f"py{dh}", tag="py")
                       for dh in range(2)]
                for hc in range(HC):
                    for dh in range(2):
                        nc.tensor.matmul(
                            pys[dh][0:qn],
                            lhsT=hT[:, hc, q0 : q0 + qn],
                            rhs=w2_sb_g[dh][:, hc],
                            start=(hc == 0),
                            stop=(hc == HC - 1),
                        )
                        n_mm += 1
                        if inject is not None and n_mm % inject_every == 0:
                            next(inject, None)
                for dh in range(2):
                    y_sb = y_pool.tile([128, 512], F32, tag="y_sb")
                    nc.vector.tensor_scalar_mul(
                        y_sb[0:qn], pys[dh][0:qn], g_sb[0:qn, qg : qg + 1]
                    )
                    nc.scalar.dma_start(
                        out=out_d[
                            t0 + q0 : t0 + q0 + qn,
                            dh * 512 : (dh + 1) * 512,
                        ],
                        in_=y_sb[0:qn],
                    )

        # Software pipeline: the x DMA for block b+1 is emitted before
        # block b's GEMMs so the PE never waits on transfers.
        # reps>1 repeats the whole sweep (timing runs only).
        offs = []
        t0 = 0
        for tb in blocks:
            offs.append((t0, tb))
            t0 += tb
        has_tail = len(offs) > 1 and offs[-1][1] < 512
        for _ in range(reps):
            xs = [emit_prep(bi, t0, tb) for bi, (t0, tb) in enumerate(offs)]
            if has_tail:
                t0_t, tb_t = offs[-1]
                hT_t = ht_pool.tile([128, HC, tb_t], mmdt, tag="hT")
                inj = tail_mm1_steps(xs[-1], tb_t, hT_t)
                big = offs[:-1]
                n_tail_mm = HC * DC
                n_big_mm = sum(2 * HC * (-(-tb // 128)) for _, tb in big)
                every = max(1, n_big_mm // n_tail_mm)
                for bi, (t0, tb) in enumerate(big):
                    hT_b = emit_mm1(xs[bi], tb)
                    emit_mm2(t0, tb, hT_b, inject=inj, inject_every=every)
                for _ in inj:
                    pass
                emit_mm2(t0_t, tb_t, hT_t)
            else:
                for bi, (t0, tb) in enumerate(offs):
                    hT_b = emit_mm1(xs[bi], tb)
                    emit_mm2(t0, tb, hT_b)

    return nc


def route(x2d, gate_w):
    """Host-side top-2 gating, mirroring the reference's eval path."""
    logits = x2d @ gate_w.T                                  # [N, E] f32
    m = logits.max(axis=1, keepdims=True)
    e = np.exp(logits - m, dtype=np.float32)
    probs = e / e.sum(axis=1, keepdims=True)
    i1 = probs.argmax(axis=1)
    n = np.arange(probs.shape[0])
    masked = probs.copy()
    masked[n, i1] = -1.0
    i2 = masked.argmax(axis=1)
    top_i = np.stack([i1, i2], axis=1).astype(np.int64)      # [N, 2]
    top_g = np.stack([probs[n, i1], probs[n, i2]], axis=1)   # [N, 2] f32
    return top_i, top_g


def make_in_maps(x2d, w1, b1, w2, top_i, top_g, C):
    np_mm = _np_mm_dt()
    N = x2d.shape[0]
    in_maps = []
    slot_pos = np.zeros((N, TOP_K), dtype=np.int64)
    for e in range(N_CORES):
        tok, slot = np.nonzero(top_i == e)
        cnt = tok.shape[0]
        assert cnt <= C, (cnt, C)
        slot_pos[tok, slot] = np.arange(cnt)
        idx = np.zeros(C, dtype=np.int64)
        idx[:cnt] = tok
        g = np.zeros(C, dtype=np.float32)
        g[:cnt] = top_g[tok, slot]
        xg = x2d[idx].astype(np_mm)                          # [C, D]
        xTc = xg.T.reshape(DC, 128, C).transpose(1, 0, 2)     # [128, DC, C]
        w1c = w1[e].astype(np_mm).reshape(DC, 128, D_HEAD).transpose(1, 0, 2)
        w1g = np.ascontiguousarray(
            w1c.reshape(128, DC, 8, 256).transpose(0, 2, 1, 3)
        )                                                     # [128, 8, DC, 256]
        w2c = w2[e].astype(np_mm).reshape(HC, 128, D_MODEL).transpose(1, 0, 2)
        w2g = np.ascontiguousarray(
            w2c.reshape(128, HC, 2, 512).transpose(0, 2, 1, 3)
        )                                                     # [128, 2, HC, 512]
        b1t = np.ascontiguousarray(b1[e].reshape(HC, 128).T)  # [128, HC]
        NQ = -(-C // 128)
        gp = np.zeros(NQ * 128, np.float32)
        gp[:C] = g
        gq = np.ascontiguousarray(gp.reshape(NQ, 128).T)     # [128, NQ]
        im = {"w1": w1g, "w2": w2g, "b1t": b1t, "g": gq}
        t0 = 0
        for bi, tb in enumerate(_blocks_for(C)):
            im[f"xT{bi}"] = np.ascontiguousarray(xTc[:, :, t0 : t0 + tb])
            t0 += tb
        in_maps.append(im)
    return in_maps, slot_pos


def kernel(x, gate_w, w1, b1, w2, b2):
    global LAST_RESULT
    x = np.asarray(x, dtype=np.float32)
    B, S, D = x.shape
    x2d = np.ascontiguousarray(x.reshape(-1, D))
    gate_w = np.asarray(gate_w, np.float32)
    w1 = np.asarray(w1, np.float32)
    b1 = np.asarray(b1, np.float32)
    w2 = np.asarray(w2, np.float32)
    b2 = np.asarray(b2, np.float32)

    top_i, top_g = route(x2d, gate_w)
    max_load = int(np.bincount(top_i.ravel(), minlength=N_EXPERTS).max())
    C = max(256, -(-max_load // 64) * 64)
    in_maps, slot_pos = make_in_maps(x2d, w1, b1, w2, top_i, top_g, C)

    nc = build_nc(C, reps=int(os.environ.get("MOE_REPS", "1")))
    # run_bass_via_pjrt serializes the module as-is; finalize() runs the
    # Bacc legalization passes (wait splitting, reg alloc) it depends on.
    nc.finalize()
    res = run_bass_kernel_spmd(nc, in_maps, core_ids=list(range(N_CORES)))
    LAST_RESULT = res

    outs = np.stack([res.results[e]["out"] for e in range(N_CORES)])  # [E, C, D]
    y = outs[top_i[:, 0], slot_pos[:, 0]] + outs[top_i[:, 1], slot_pos[:, 1]]
    y += (top_g[:, :, None] * b2[top_i]).sum(axis=1)
    return y.astype(np.float32).reshape(B, S, D)


# revision 17
# speedup vs baseline: 1.2588x; 1.2588x over previous
"""Trainium2 Bass kernel for nn_MoE (moe_routing).

Strategy: expert parallelism with host-side dispatch/combine (the
"all-to-all" of the sharding hint, performed while sharding the full
inputs).  The host computes the top-2 gating (cheap: a 4096x1024 @
1024x8 matmul + softmax), gathers each expert's routed tokens into a
padded capacity-C buffer, and core e computes, densely over its C
gathered tokens,

    out_e = gate[t] * (gelu(x_t @ w1[e] + b1[e]) @ w2[e])

The host combine gathers each token's two expert rows and adds the
(gate-weighted) b2 bias terms.  This is exact w.r.t. the reference's
dense-per-expert formulation: every routed (token, expert) pair is
computed; padding rows carry gate 0.

Matmuls run in bf16 (full-rate PE mode, fp32 PSUM accumulation) --
tolerance-checked far under the 2e-2 gate.  Set MOE_F32R_MM=1 for
float32r operands instead (same PE rate at moving-dim >= 256, 2x DMA).
"""

import os
from contextlib import ExitStack

import numpy as np

import concourse.bass as bass
from concourse import bacc
import concourse.mybir as mybir
import concourse.tile as tile
from concourse.bass_utils import run_bass_kernel_spmd

F32 = mybir.dt.float32
BF16 = mybir.dt.bfloat16
F32R = mybir.dt.float32r
AF = mybir.ActivationFunctionType

D_MODEL = 1024
D_HEAD = 2048
N_EXPERTS = 8
TOP_K = 2
N_CORES = 8

DC = D_MODEL // 128      # d_model chunks of 128
HC = D_HEAD // 128       # d_head chunks of 128

LAST_RESULT = None       # BassKernelResults of the most recent run (for test.py)


def _mm_dt():
    return F32R if os.environ.get("MOE_F32R_MM") == "1" else BF16


def _np_mm_dt():
    import ml_dtypes
    return np.float32 if os.environ.get("MOE_F32R_MM") == "1" else ml_dtypes.bfloat16


def _blocks_for(C):
    """Decompose capacity C (multiple of 64) into token blocks of 512
    plus one 64..448 tail."""
    blocks = [512] * (C // 512)
    if C % 512:
        blocks.append(C % 512)
    return blocks


def build_nc(C, reps=1):
    """Build the single-core SPMD Bass program for capacity C tokens."""
    mmdt = _mm_dt()
    nc = bacc.Bacc()
    blocks = _blocks_for(C)
    NQ = -(-C // 128)        # 128-token quarters overall (last may be half)

    xT_ds = [
        nc.declare_dram_parameter(f"xT{bi}", [128, DC, tb], mmdt, isOutput=False)
        for bi, tb in enumerate(blocks)
    ]
    w1_d = nc.declare_dram_parameter("w1", [128, 8, DC, 256], mmdt, isOutput=False)
    w2_d = nc.declare_dram_parameter("w2", [128, 2, HC, 512], mmdt, isOutput=False)
    b1t_d = nc.declare_dram_parameter("b1t", [128, HC], F32, isOutput=False)
    g_d = nc.declare_dram_parameter("g", [128, NQ], F32, isOutput=False)
    out_d = nc.declare_dram_parameter("out", [C, D_MODEL], F32, isOutput=True)

    with tile.TileContext(nc) as tc, ExitStack() as ctx:
        singles = ctx.enter_context(tc.tile_pool(name="singles", bufs=1))
        xt_pool = ctx.enter_context(tc.tile_pool(name="xt", bufs=2))
        ht_pool = ctx.enter_context(tc.tile_pool(name="ht", bufs=2))
        y_pool = ctx.enter_context(tc.tile_pool(name="yb", bufs=4))
        ps_h = ctx.enter_context(tc.tile_pool(name="ps_h", bufs=3, space="PSUM"))
        ps_y = ctx.enter_context(tc.tile_pool(name="ps_y", bufs=2, space="PSUM"))

        b1t_sb = singles.tile([128, HC], F32)
        g_sb = singles.tile([128, NQ], F32)
        nc.gpsimd.dma_start(out=b1t_sb, in_=b1t_d[:])
        nc.gpsimd.dma_start(out=g_sb, in_=g_d[:])

        # w1 as 4 independent hc-group tiles so block 0's mm1 can begin
        # before the full weight load has landed; w2 as 2 dh-half tiles in
        # the order mm2 consumes them.  All weight traffic rides the gpsimd
        # queues so x/y DMAs on nc.sync are never stuck behind it.
        HCG = HC // 8
        w1_sb_g = []
        for gi in range(8):
            t = singles.tile([128, DC, HCG * 128], mmdt, name=f"w1g{gi}")
            w1_sb_g.append(t)
            nc.scalar.dma_start(out=t, in_=w1_d[:, gi])
        w2_sb_g = []
        for dh in range(2):
            t = singles.tile([128, HC, 512], mmdt, name=f"w2g{dh}")
            w2_sb_g.append(t)
            nc.scalar.dma_start(out=t, in_=w2_d[:, dh])

        def emit_prep(blk_i, t0, tb):
            xT = xt_pool.tile([128, DC, tb], mmdt, tag="xT")
            nc.sync.dma_start(out=xT, in_=xT_ds[blk_i][:])
            return xT

        def emit_mm1(xT, tb):
            hT = ht_pool.tile([128, HC, tb], mmdt, tag="hT")
            for hc in range(HC):
                ph = ps_h.tile([128, tb], F32, tag="ph")
                w1t = w1_sb_g[hc // HCG]
                hcl = hc % HCG
                for dc in range(DC):
                    nc.tensor.matmul(
                        ph,
                        lhsT=w1t[:, dc, hcl * 128 : (hcl + 1) * 128],
                        rhs=xT[:, dc],
                        start=(dc == 0),
                        stop=(dc == DC - 1),
                    )
                nc.scalar.activation(
                    hT[:, hc], ph, AF.Gelu, bias=b1t_sb[:, hc : hc + 1]
                )
            return hT

        def emit_mm2(t0, tb, hT):
            # quarters of 128 tokens, plus one 64-token half-quarter when
            # tb % 128 == 64 (psum/output partition dim 64).
            quarters = [(q * 128, 128) for q in range(tb // 128)]
            if tb % 128:
                quarters.append((tb - tb % 128, tb % 128))
            for (q0, qn) in quarters:
                qg = (t0 + q0) // 128
                pys = [ps_y.tile([128, 512], F32, name=f"py{dh}", tag="py")
                       for dh in range(2)]
                for hc in range(HC):
                    for dh in range(2):
                        nc.tensor.matmul(
                            pys[dh][0:qn],
                            lhsT=hT[:, hc, q0 : q0 + qn],
                            rhs=w2_sb_g[dh][:, hc],
                            start=(hc == 0),
                            stop=(hc == HC - 1),
                        )
                for dh in range(2):
                    y_sb = y_pool.tile([128, 512], F32, tag="y_sb")
                    nc.vector.tensor_scalar_mul(
                        y_sb[0:qn], pys[dh][0:qn], g_sb[0:qn, qg : qg + 1]
                    )
                    nc.scalar.dma_start(
                        out=out_d[
                            t0 + q0 : t0 + q0 + qn,
                            dh * 512 : (dh + 1) * 512,
                        ],
                        in_=y_sb[0:qn],
                    )

        # Software pipeline: the x DMA for block b+1 is emitted before
        # block b's GEMMs so the PE never waits on transfers.
        # reps>1 repeats the whole sweep (timing runs only).
        offs = []
        t0 = 0
        for tb in blocks:
            offs.append((t0, tb))
            t0 += tb
        for _ in range(reps):
            xT_b = emit_prep(0, offs[0][0], offs[0][1])
            for bi, (t0, tb) in enumerate(offs):
                if bi + 1 < len(offs):
                    xT_n = emit_prep(bi + 1, offs[bi + 1][0], offs[bi + 1][1])
                hT_b = emit_mm1(xT_b, tb)
                emit_mm2(t0, tb, hT_b)
                if bi + 1 < len(offs):
                    xT_b = xT_n

    return nc


def route(x2d, gate_w):
    """Host-side top-2 gating, mirroring the reference's eval path."""
    logits = x2d @ gate_w.T                                  # [N, E] f32
    m = logits.max(axis=1, keepdims=True)
    e = np.exp(logits - m, dtype=np.float32)
    probs = e / e.sum(axis=1, keepdims=True)
    i1 = probs.argmax(axis=1)
    n = np.arange(probs.shape[0])
    masked = probs.copy()
    masked[n, i1] = -1.0
    i2 = masked.argmax(axis=1)
    top_i = np.stack([i1, i2], axis=1).astype(np.int64)      # [N, 2]
    top_g = np.stack([probs[n, i1], probs[n, i2]], axis=1)   # [N, 2] f32
    return top_i, top_g


def make_in_maps(x2d, w1, b1, w2, top_i, top_g, C):
    np_mm = _np_mm_dt()
    N = x2d.shape[0]
    in_maps = []
    slot_pos = np.zeros((N, TOP_K), dtype=np.int64)
    for e in range(N_CORES):
        tok, slot = np.nonzero(top_i == e)
        cnt = tok.shape[0]
        assert cnt <= C, (cnt, C)
        slot_pos[tok, slot] = np.arange(cnt)
        idx = np.zeros(C, dtype=np.int64)
        idx[:cnt] = tok
        g = np.zeros(C, dtype=np.float32)
        g[:cnt] = top_g[tok, slot]
        xg = x2d[idx].astype(np_mm)                          # [C, D]
        xTc = xg.T.reshape(DC, 128, C).transpose(1, 0, 2)     # [128, DC, C]
        w1c = w1[e].astype(np_mm).reshape(DC, 128, D_HEAD).transpose(1, 0, 2)
        w1g = np.ascontiguousarray(
            w1c.reshape(128, DC, 8, 256).transpose(0, 2, 1, 3)
        )                                                     # [128, 8, DC, 256]
        w2c = w2[e].astype(np_mm).reshape(HC, 128, D_MODEL).transpose(1, 0, 2)
        w2g = np.ascontiguousarray(
            w2c.reshape(128, HC, 2, 512).transpose(0, 2, 1, 3)
        )                                                     # [128, 2, HC, 512]
        b1t = np.ascontiguousarray(b1[e].reshape(HC, 128).T)  # [128, HC]
        NQ = -(-C // 128)
        gp = np.zeros(NQ * 128, np.float32)
        gp[:C] = g
        gq = np.ascontiguousarray(gp.reshape(NQ, 128).T)     # [128, NQ]
        im = {"w1": w1g, "w2": w2g, "b1t": b1t, "g": gq}
        t0 = 0
        for bi, tb in enumerate(_blocks_for(C)):
            im[f"xT{bi}"] = np.ascontiguousarray(xTc[:, :, t0 : t0 + tb])
            t0 += tb
        in_maps.append(im)
    return in_maps, slot_pos


def kernel(x, gate_w, w1, b1, w2, b2):
    global LAST_RESULT
    x = np.asarray(x, dtype=np.float32)
    B, S, D = x.shape
    x2d = np.ascontiguousarray(x.reshape(-1, D))
    gate_w = np.asarray(gate_w, np.float32)
    w1 = np.asarray(w1, np.float32)
    b1 = np.asarray(b1, np.float32)
    w2 = np.asarray(w2, np.float32)
    b2 = np.asarray(b2, np.float32)

    top_i, top_g = route(x2d, gate_w)
    max_load = int(np.bincount(top_i.ravel(), minlength=N_EXPERTS).max())
    C = max(256, -(-max_load // 64) * 64)
    in_maps, slot_pos = make_in_maps(x2d, w1, b1, w2, top_i, top_g, C)

    nc = build_nc(C, reps=int(os.environ.get("MOE_REPS", "1")))
    # run_bass_via_pjrt serializes the module as-is; finalize() runs the
    # Bacc legalization passes (wait splitting, reg alloc) it depends on.
    nc.finalize()
    res = run_bass_kernel_spmd(nc, in_maps, core_ids=list(range(N_CORES)))
    LAST_RESULT = res

    outs = np.stack([res.results[e]["out"] for e in range(N_CORES)])  # [E, C, D]
    y = outs[top_i[:, 0], slot_pos[:, 0]] + outs[top_i[:, 1], slot_pos[:, 1]]
    y += (top_g[:, :, None] * b2[top_i]).sum(axis=1)
    return y.astype(np.float32).reshape(B, S, D)
